# revision 1
# baseline (speedup 1.0000x reference)
"""Trainium2 Bass kernel for nn_ComposedCliffordSteerableKernel.

Computation (see reference): for each of 16x16 (m, n) block pairs, a tiny
3D conv (8,8,7^3) x (8,8,7^3) -> (8,8,7^3) with SAME padding, then
elementwise * shell * factor.

Both conv operands depend on the pair, so each pair is an independent
[M=8, K=8, N] matmul per spatial tap -- far too small for the 128x128 PE
array on its own.  Two packings are implemented:

- "f32r"/"f32" (_build_nc): per m-block (8 output rows), one 128x128
  block-diagonal matmul per tap: contraction partitions (n,j) = 16 pairs
  x 8 input blades, output partitions (n,q), free dim = spatial output
  positions of one batch-blade p (N=392, w padded to 8 for FP32R's even
  innermost-run rule).  8 PSUM banks (one per p) accumulate all 343
  taps.  float32r gives single-pass fp32 (1 cycle/row at N>=256) at
  ~tf32 precision (measured 1.4e-4 rel).

- "*t16" (_build_nc_t16): the PE is packed as 16 independent 32x32
  tiles.  Tile (row 32g, col 32c) contracts pair-group g (4 pairs) and
  writes PSUM strip c; pairing c = (g + t) % 4 over tap-classes
  t = lin % 4 uses all 16 tiles and quadruples useful MAC rate vs the
  block-diagonal scheme.  Per output depth od, 4 PSUM banks (one per
  class, od-parity double-buffered) accumulate the taps; output strip s
  is then sum over t of bank_t[strip (s+t)%4] (partition-crossed DVE
  adds).  Zero-contribution (od,kd) pairs are skipped and oh is
  restricted to its valid window (~1.75x fewer MACs).
  Multi-pass modes sweep pass-major so consecutive PE instructions hit
  different tiles (PE matmul starts are pc-monotone; per-tile pass
  chains would collapse the packing to ~1.5x).
  dtypes: "fp16t16" 1-pass fp16 (~3e-4 rel); "bf16t16" 1-pass bf16
  (~2e-3); "bf16x3t16" hi/lo-split 3-pass bf16 (~4e-6, fp32-grade).

k1 is held transposed (columns -> partitions) and zero-padded to
(13,13,14) so every tap is just an AP window offset; weights are
DMA-scattered into block-diagonal SBUF tiles whose off-diagonal zeros
persist from a one-time fill.  Sharding: core c takes output row-blocks
2c and 2c+1; no inter-core communication.
"""

import sys

for _p in ("/opt/trn_rl_repo",):
    if _p not in sys.path:
        sys.path.insert(0, _p)

import numpy as np

NB = 8
KS = 7
S3 = KS * KS * KS          # 343
WPAD = KS + 1              # 8 (even innermost run for fp32r)
SP = KS * KS * WPAD        # 392 psum free size per batch-blade
DPAD, HPAD, WPAD2 = 13, 13, 14
PADVOL = DPAD * HPAD * WPAD2   # 2366 per batch-blade in k1T
N_CORES = 8
M_PER_CORE = 2             # m-blocks per core

# All HW-validated (rel err to reference / notes):
#   "bf16x3t16": 4.3e-6, 16-tile packed PE, 3-pass hi/lo bf16  <- default
#   "fp16t16":   2.9e-4, 16-tile packed PE, fastest
#   "f32r":      1.4e-4, single 128x128 block-diag matmuls
#   "f32":       exact fp32 (4 cycles/row), slowest
MODE = "bf16x3t16"

_CACHE = {}

SPT = KS * WPAD * NB       # 448: T16 psum free per od: (p, oh, ow8)


def _build_nc(mode):
    import concourse.bass as bass
    import concourse.tile as tile
    from concourse import bacc, mybir

    f32 = mybir.dt.float32
    f32r = mybir.dt.float32r
    mult = mybir.AluOpType.mult

    nc = bacc.Bacc("TRN2", target_bir_lowering=False, debug=False)

    # k1 arrives host-padded: [16 rows, 128 cols, 13*13*14] with the 7^3
    # interior at [3:10,3:10,3:10] (f32r tiles cannot be memset, so the
    # zero padding comes in via the cast DMA)
    k1 = nc.dram_tensor(
        "k1pad", [M_PER_CORE * NB, 128, PADVOL], f32, kind="ExternalInput"
    )
    k2 = nc.dram_tensor("k2", [M_PER_CORE * NB, 128, S3], f32, kind="ExternalInput")
    shell = nc.dram_tensor(
        "shell", [M_PER_CORE * NB, 128, SP], f32, kind="ExternalInput"
    )
    factor = nc.dram_tensor("factor", [128, 1], f32, kind="ExternalInput")
    zeros = nc.dram_tensor(
        "zeros", [128, 128 * KS * KS], f32, kind="ExternalInput"
    )
    out = nc.dram_tensor("out", [M_PER_CORE * NB, 128, SP], f32, kind="ExternalOutput")

    mm_dt = f32r if mode == "f32r" else f32

    with tile.TileContext(nc) as tc:
        with (
            tc.tile_pool(name="persist", bufs=1) as persist,
            tc.tile_pool(name="io", bufs=2) as io,
            tc.tile_pool(name="ps", bufs=1, space="PSUM") as pspool,
        ):
            # k1 transposed + zero padded: [(n,j)=128, p=8, 13, 13, 14]
            # stored as float32r so fp32r matmuls accept it (DMA casts)
            k1t = persist.tile([128, NB, DPAD, HPAD, WPAD2], mm_dt, tag="k1t")

            # two weight chunk slots, each one kd-plane of 49 taps:
            # [(n,j)=128, (n,q)=128, tap=49] (taps contiguous so the k2
            # DMA has a stride-1 final dim); zeros off the diagonal persist
            # from a one-time cast-DMA fill from the zeros input
            wslots = []
            for i in range(2):
                w = persist.tile([128, 128, KS * KS], mm_dt, tag=f"w{i}", name=f"w{i}")
                nc.gpsimd.dma_start(
                    out=w.rearrange("c a t -> c (a t)"), in_=zeros[:, :]
                )
                wslots.append(w)

            fac = persist.tile([128, 1], f32, tag="fac")
            nc.sync.dma_start(out=fac[:, :], in_=factor[:, :])

            psum = [
                pspool.tile([128, SP], f32, tag=f"pp{p}", name=f"pp{p}")
                for p in range(NB)
            ]

            for m in range(M_PER_CORE):
                # load k1 block (host-padded, transposed into partitions);
                # one contiguous cast DMA per batch-blade p
                for p in range(NB):
                    nc.gpsimd.dma_start(
                        out=k1t[:, p, :, :, :],
                        in_=k1[m * NB + p, :, :],
                    )

                # shell for this m (host pre-padded w->8, so contiguous),
                # pre-scaled by factor
                sh = io.tile([128, NB, SP], f32, tag="shell")
                nc.sync.dma_start(
                    out=sh[:, :, :],
                    in_=shell[m * NB:(m + 1) * NB, :, :].rearrange("p c s -> c p s"),
                )
                shf = io.tile([128, NB, SP], f32, tag="shellf")
                nc.vector.tensor_scalar_mul(shf[:, :, :], sh[:, :, :], fac[:, 0:1])

                for kd in range(KS):
                    w = wslots[kd % 2]
                    # load this kd-plane's 16 diagonal blocks:
                    # w[n*8+j, n*8+q, t] = k2[m*8+q, n*8+j, kd*49+t]
                    for n in range(16):
                        nc.gpsimd.dma_start(
                            out=w[n * NB:(n + 1) * NB, n * NB:(n + 1) * NB, :],
                            in_=k2[
                                m * NB:(m + 1) * NB,
                                n * NB:(n + 1) * NB,
                                kd * KS * KS:(kd + 1) * KS * KS,
                            ].rearrange("q j t -> j q t"),
                        )
                    for kh in range(KS):
                        for kw in range(KS):
                            t = kh * KS + kw
                            lhsT = w[:, :, t]
                            first = kd == 0 and t == 0
                            last = kd == KS - 1 and t == KS * KS - 1
                            for p in range(NB):
                                rhs = k1t[
                                    :, p, kd:kd + KS, kh:kh + KS, kw:kw + WPAD
                                ]
                                nc.tensor.matmul(
                                    psum[p][:, :],
                                    lhsT,
                                    rhs,
                                    start=first,
                                    stop=last,
                                )

                # evacuate: out = psum * factor * shell  (shell already
                # carries factor), then store
                ost = io.tile([128, NB, SP], f32, tag="ost")
                for p in range(NB):
                    nc.vector.tensor_mul(
                        ost[:, p, :], psum[p][:, :], shf[:, p, :]
                    )
                nc.sync.dma_start(
                    out=out[m * NB:(m + 1) * NB, :, :].rearrange("p c s -> c p s"),
                    in_=ost[:, :, :],
                )
    nc.compile()
    return nc


def _build_nc_t16(mode):
    """16x 32x32 PE-tile variant (bf16/fp16).

    Per m-block, per output depth od (7), accumulate the valid taps into
    4 PSUM banks (one per tap-class t = lin%4), double-buffered by od
    parity.  Tile (row 32g, col 32c) contracts pair-group g (SBUF
    partitions 32g..32g+31 of k1t) and writes PSUM partitions 32c;
    pairing c = (g+t)%4 uses all 16 tiles.  Output strip s is then
    sum over t of bank_t[strip (s+t)%4]; partition rotation goes through
    SBUF->SBUF DMA (engines cannot cross partitions).

    Multi-pass modes emit pass-major sweeps: PE matmuls start in program
    order, so back-to-back passes on the SAME tile would serialize and
    collapse the 16-tile concurrency; sweeping all (tap, g) per pass
    keeps consecutive instructions on different tiles.

    psum bank free layout is (oh, p, ow) with ow=7 (no fp32r evenness
    rule here), so an oh-window slice stays a contiguous slab (the sim's
    matmul needs 2D-flattenable psum dst APs).
    """
    import concourse.tile as tile
    from concourse import bacc, mybir

    f32 = mybir.dt.float32
    bf16 = (mybir.dt.float16 if mode == "fp16t16" else mybir.dt.bfloat16)
    npass = 3 if mode == "bf16x3t16" else 1
    SPT7 = KS * KS * NB            # 392: (oh, p, ow7)
    S2 = KS * KS

    nc = bacc.Bacc("TRN2", target_bir_lowering=False, debug=False)

    names = ["h"] if npass == 1 else ["h", "l"]
    k1d = {
        s: nc.dram_tensor(f"k1{s}", [M_PER_CORE * NB, 128, S3], bf16,
                          kind="ExternalInput")
        for s in names
    }
    k2d = {
        s: nc.dram_tensor(f"k2{s}", [M_PER_CORE * NB, 128, S3], bf16,
                          kind="ExternalInput")
        for s in names
    }
    shell = nc.dram_tensor(
        "shell", [M_PER_CORE * NB, 128, S3], f32, kind="ExternalInput"
    )
    factor = nc.dram_tensor("factor", [128, 1], f32, kind="ExternalInput")
    out = nc.dram_tensor("out", [M_PER_CORE * NB, 128, S3], f32,
                         kind="ExternalOutput")

    # (weight-piece, k1-piece) per pass: h*h + h*l + l*h
    passes = [("h", "h")] if npass == 1 else [("h", "h"), ("h", "l"), ("l", "h")]

    with tile.TileContext(nc) as tc:
        with (
            tc.tile_pool(name="persist", bufs=1) as persist,
            tc.tile_pool(name="io", bufs=2) as io,
            tc.tile_pool(name="ps", bufs=1, space="PSUM") as pspool,
        ):
            # k1t: (d, h) padding is never read (the kd-skip keeps
            # od+kd in the interior and the oh-window keeps oh+kh in the
            # interior), so only w carries the zero halo: 9KB/partition
            # per piece instead of 35KB -- leaves room to double-buffer
            # k1t AND weights across m-blocks (no m-boundary PE stall)
            k1t = {
                (s, i): persist.tile([128, NB, KS, KS, DPAD], bf16,
                                     tag=f"k1t{s}{i}", name=f"k1t{s}{i}")
                for s in names for i in range(2)
            }
            for tile_ in k1t.values():
                nc.vector.memset(tile_[:, :, :, :, :], 0.0)

            # weights: [128=(g,nsub,j), 32=(nsub,q), 343 taps] per piece
            nwslot = 2
            wt = {}
            for s in names:
                for i in range(nwslot):
                    w = persist.tile([128, 32, S3], bf16,
                                     tag=f"wt{s}{i}", name=f"wt{s}{i}")
                    nc.vector.memset(w[:, :, :], 0.0)
                    wt[(s, i)] = w

            fac = persist.tile([128, 1], f32, tag="fac")
            nc.sync.dma_start(out=fac[:, :], in_=factor[:, :])

            # psum: [od-parity][class] -> [128, 392] (allocated 400 wide
            # so 32-partition strip offsets stay 2KB-bank aligned:
            # 32*400*4 % 2048 == 0)
            psumb = [
                [
                    pspool.tile([128, 400], f32, tag=f"pb{par}{t}",
                                name=f"pb{par}{t}")[:, 0:SPT7]
                    for t in range(4)
                ]
                for par in range(2)
            ]
            # valid-window skipping leaves some psum elements unwritten
            # in a round (their true partial is 0); a one-time zero fill
            # keeps those reads defined
            for par in range(2):
                for t in range(4):
                    nc.vector.memset(psumb[par][t][:, :], 0.0)

            for m in range(M_PER_CORE):
                k1m = {s: k1t[(s, m % 2)] for s in names}
                for s in names:
                    for p in range(NB):
                        src_p = k1d[s][m * NB + p, :, :].rearrange(
                            "c (d h w) -> c d h w", d=KS, h=KS, w=KS
                        )
                        for d in range(KS):
                            nc.sync.dma_start(
                                out=k1m[s][:, p, d, :, 3:3 + KS],
                                in_=src_p[:, d, :, :],
                            )
                wm = {s: wt[(s, m % nwslot)] for s in names}
                for s in names:
                    for n in range(16):
                        nc.sync.dma_start(
                            out=wm[s][n * NB:(n + 1) * NB,
                                      (n % 4) * NB:(n % 4 + 1) * NB, :],
                            in_=k2d[s][
                                m * NB:(m + 1) * NB, n * NB:(n + 1) * NB, :
                            ].rearrange("q j t -> j q t"),
                        )

                shf = io.tile([128, NB, S3], f32, tag="shell")
                nc.sync.dma_start(
                    out=shf[:, :, :],
                    in_=shell[m * NB:(m + 1) * NB, :, :].rearrange("p c s -> c p s"),
                )
                nc.vector.tensor_scalar_mul(shf[:, :, :], shf[:, :, :], fac[:, 0:1])

                ost = io.tile([128, NB, KS, KS, KS], f32, tag="ost")

                for od in range(KS):
                    par = od % 2
                    # valid windows: contributions are zero unless the
                    # padded read index lands in the 7^3 interior [3,10)
                    kds = [kd for kd in range(KS) if 3 <= od + kd <= 9]
                    # each class t starts with a full-oh tap (kh=3; class
                    # of (kd,3,kw) is (kd+1+kw)%4) so the accumulation
                    # group's first matmul covers the whole bank
                    firsts = []
                    for t in range(4):
                        kd0 = kds[0]
                        kw0 = (t - kd0 - 1) % 4
                        firsts.append(kd0 * S2 + 3 * KS + kw0)
                    assert sorted(l % 4 for l in firsts) == [0, 1, 2, 3]
                    ordered = firsts + [
                        lin
                        for kd in kds
                        for lin in range(kd * S2, (kd + 1) * S2)
                        if lin not in set(firsts)
                    ]
                    last_lin_od = {t: max(l for l in ordered if l % 4 == t)
                                   for t in range(4)}
                    for ip, (ws, ks) in enumerate(passes):
                        for i, lin in enumerate(ordered):
                            kd, r = divmod(lin, S2)
                            kh, kw = divmod(r, KS)
                            oh0, oh1 = max(0, 3 - kh), min(KS, 10 - kh)
                            t = lin % 4
                            first = ip == 0 and i < 4
                            last = ip == npass - 1 and lin == last_lin_od[t]
                            for g in range(4):
                                c = (g + t) % 4
                                dst = psumb[par][t][
                                    32 * c:32 * c + 32, :
                                ].rearrange(
                                    "c (oh p ow) -> c oh p ow", oh=KS, p=NB,
                                )[:, oh0:oh1, :, :]
                                rhs = k1m[ks][
                                    32 * g:32 * g + 32, :,
                                    od + kd - 3,
                                    kh + oh0 - 3:kh + oh1 - 3,
                                    kw:kw + KS,
                                ].transpose([0, 2, 1, 3])  # (oh, p, ow)
                                nc.tensor.matmul(
                                    dst,
                                    wm[ws][32 * g:32 * g + 32, :, lin],
                                    rhs,
                                    start=first,
                                    stop=last,
                                    tile_position=(32 * g, 32 * c),
                                    # sim group-check is per 2KB
                                    # zero-region; per-strip groups are
                                    # safe on HW (num_active_cols=32)
                                    skip_group_check=True,
                                )
                    # combine rotated partials into ost[:, :, od, :, :].
                    # bank 0 is strip-aligned (c = g for t = 0) and is
                    # read from PSUM directly; banks 1-3 go through an
                    # aligned DVE evacuation then a partition-rotating
                    # SBUF->SBUF DMA.
                    ev = {
                        t: io.tile([128, SPT7], f32, tag=f"ev{t}",
                                   name=f"ev{t}")
                        for t in range(1, 4)
                    }
                    for t in range(1, 4):
                        nc.vector.tensor_copy(ev[t][:, :], psumb[par][t][:, :])
                    rt = {}
                    for t in range(1, 4):
                        r = io.tile([128, SPT7], f32, tag=f"rt{t}",
                                    name=f"rt{t}")
                        sh4 = 32 * t
                        nc.sync.dma_start(
                            out=r[0:128 - sh4, :], in_=ev[t][sh4:128, :]
                        )
                        nc.sync.dma_start(
                            out=r[128 - sh4:128, :], in_=ev[t][0:sh4, :]
                        )
                        rt[t] = r
                    o_sl = ost[:, :, od, :, :]
                    fix = lambda ap: ap.rearrange(
                        "c (oh p ow) -> c p oh ow", oh=KS, p=NB
                    )
                    nc.vector.tensor_add(
                        o_sl, fix(psumb[par][0][:, :]), fix(rt[1][:, :])
                    )
                    nc.vector.tensor_add(o_sl, o_sl, fix(rt[2][:, :]))
                    nc.vector.tensor_add(o_sl, o_sl, fix(rt[3][:, :]))

                ostf = ost.rearrange("c p a b w -> c p (a b w)")
                nc.vector.tensor_mul(ostf[:, :, :], ostf[:, :, :], shf[:, :, :])
                nc.sync.dma_start(
                    out=out[m * NB:(m + 1) * NB, :, :].rearrange("p c s -> c p s"),
                    in_=ostf[:, :, :],
                )
    nc.compile()
    return nc


def _get_nc(mode=None):
    if mode is None:
        mode = MODE
    if mode not in _CACHE:
        if mode in ("bf16t16", "bf16x3t16", "fp16t16"):
            _CACHE[mode] = _build_nc_t16(mode)
        else:
            _CACHE[mode] = _build_nc(mode)
    return _CACHE[mode]


def _make_in_maps(k1, k2, shell, factor, mode=None):
    import ml_dtypes

    if mode is None:
        mode = MODE

    k1 = np.ascontiguousarray(k1.reshape(128, 128, S3), np.float32)
    k2 = np.ascontiguousarray(k2.reshape(128, 128, S3), np.float32)
    if mode in ("f32r", "f32"):
        shell_p = np.zeros((128, 128, KS, KS, WPAD), np.float32)
        shell_p[..., :KS] = shell.reshape(128, 128, KS, KS, KS)
        shell_p = shell_p.reshape(128, 128, SP)
    else:
        shell_p = np.ascontiguousarray(shell.reshape(128, 128, S3), np.float32)
    fac = np.full((128, 1), np.float32(factor.reshape(-1)[0]), np.float32)
    rows = M_PER_CORE * NB

    common = {"shell": shell_p, "factor": fac}
    if mode in ("f32r", "f32"):
        k1_pad = np.zeros((128, 128, DPAD, HPAD, WPAD2), np.float32)
        k1_pad[:, :, 3:3 + KS, 3:3 + KS, 3:3 + KS] = k1.reshape(
            128, 128, KS, KS, KS
        )
        k1_pad = k1_pad.reshape(128, 128, PADVOL)
        zeros = np.zeros((128, 128 * KS * KS), np.float32)
        per_full = {"k1pad": k1_pad, "k2": k2, **common}
        shared = {"zeros": zeros}
    else:
        bf = np.float16 if mode == "fp16t16" else ml_dtypes.bfloat16
        k1h = k1.astype(bf)
        k2h = k2.astype(bf)
        per_full = {"k1h": k1h, "k2h": k2h, **common}
        if mode == "bf16x3t16":
            per_full["k1l"] = (k1 - k1h.astype(np.float32)).astype(bf)
            per_full["k2l"] = (k2 - k2h.astype(np.float32)).astype(bf)
        shared = {}

    maps = []
    for c in range(N_CORES):
        m = {k: v[c * rows:(c + 1) * rows] for k, v in per_full.items()
             if k != "factor"}
        m["factor"] = fac
        m.update(shared)
        maps.append(m)
    return maps


def _gather(results):
    outs = [np.asarray(r["out"]) for r in results]
    full = np.concatenate(outs, axis=0)          # (128, 128, 392|343)
    if full.shape[-1] == SP:  # f32r/f32 path: strip the ow pad
        full = full.reshape(128, 128, KS, KS, WPAD)[..., :KS]
        return np.ascontiguousarray(full)
    return full.reshape(128, 128, KS, KS, KS)


def kernel(k1, k2, shell, factor, _trace=False):
    from concourse.bass_utils import run_bass_kernel_spmd

    nc = _get_nc(MODE)
    in_maps = _make_in_maps(
        np.asarray(k1), np.asarray(k2), np.asarray(shell), np.asarray(factor),
        mode=MODE,
    )
    try:
        res = run_bass_kernel_spmd(
            nc, in_maps, core_ids=list(range(N_CORES)), trace=_trace
        )
    except ModuleNotFoundError:
        # no NTFF profiling hook in this container; run without trace
        res = run_bass_kernel_spmd(
            nc, in_maps, core_ids=list(range(N_CORES)), trace=False
        )
    out = _gather(res.results)
    if _trace:
        return out, res
    return out



# revision 8
# speedup vs baseline: 47.5980x; 47.5980x over previous
"""Trainium2 Bass kernel for nn_ComposedCliffordSteerableKernel.

Computation (see reference): for each of 16x16 (m, n) block pairs, a tiny
3D conv (8,8,7^3) x (8,8,7^3) -> (8,8,7^3) with SAME padding, then
elementwise * shell * factor:

  out[m8+p, n8+q, od,oh,ow] =
      sum_{j,kd,kh,kw} k2[m8+q, n8+j, kd,kh,kw]
                     * k1[m8+p, n8+j, od+kd-3, oh+kh-3, ow+kw-3]

The cost model charges a matmul `output_free_size * cycles_per_row`
regardless of how many PE rows/columns are used, so the winning layout
maximizes contraction+output partitions per instruction and minimizes
streamed rows.  This kernel uses a *Toeplitz-in-depth* packing:

- PSUM partitions   = (nb, q, od)  : pair-in-duo, out blade, out depth = 112
- contraction rows  = (nb, j, id)  : pair-in-duo, in blade, abs. in depth = 112
- chunk loop        = (kh, kw)     : 49 accumulating matmuls per (m, duo)
- streamed free dim = (oh, p, ow)  : <= 392, oh restricted to the valid
                      window per kh (sum_kh win(kh) = 37 instead of 49)

The kd contraction is absorbed into a host-precomputed block-diagonal
Toeplitz weight tile w[(nb,j,id),(nb,q,od)] = k2[q,j,id-od+3,kh,kw]
(zero off the n-diagonal and off the |id-od|<=3 band).  rhs is plain k1
with (nb,j,id) on partitions and (p,ih,iw) in-partition (w zero-padded to
13 so iw=ow+kw-3 is always in range; ih stays interior thanks to the oh
window).  Chunk (kh=3,kw=3) runs first: its oh window is full, so the
accumulation group's start=True matmul covers the whole PSUM tile.

fp16 operands (measured ~3e-4 rel err vs the 2e-2 gate; PSUM accumulates
fp32).  shell*factor is folded host-side and applied during the PSUM
evacuation multiply; outputs return as fp16 and are unpacked on host.

Charged PE rows: 2m * 8duo * sum_{kh,kw} 8p*win(kh)*7ow = 232,064
(~97us at 2.4GHz) vs the previous 16-tile kernel's 12.9M (~5.4ms).

Sharding: core c takes output row-blocks 2c and 2c+1 (16 of 128 rows);
no inter-core communication.
"""

import sys

for _p in ("/opt/trn_rl_repo",):
    if _p not in sys.path:
        sys.path.insert(0, _p)

import numpy as np

NB = 8
KS = 7
N_CORES = 8
M_PER_CORE = 2
DUOS = 8                   # n-pair duos per m-block
PART = 112                 # (nb2, j8, id7) = (nb2, q8, od7)
SPF = KS * NB * KS         # 392 free: (oh, p, ow)
WPAD = 13                  # iw = ow + kw - 3 in [-3, 9] -> pad w by 3+3
CH = KS * KS               # 49 (kh, kw) chunks

# chunk order: (3,3) first (full oh window -> start=True covers the
# whole psum tile), rest lexicographic; last chunk carries stop=True
CHUNKS = [(3, 3)] + [
    (kh, kw) for kh in range(KS) for kw in range(KS) if (kh, kw) != (3, 3)
]

MODE = "toep16"

_CACHE = {}


def _build_nc(mode):
    import concourse.tile as tile
    from concourse import bacc, mybir

    f16 = mybir.dt.float16
    f32 = mybir.dt.float32

    nc = bacc.Bacc("TRN2", target_bir_lowering=False, debug=False)

    k1r = nc.dram_tensor(
        "k1r", [M_PER_CORE, DUOS, PART, NB * KS * WPAD], f16,
        kind="ExternalInput"
    )
    # full block-diagonal Toeplitz incl. zeros: walrus requires a 2D
    # weights AP, so the op columns must be contiguous per chunk
    wt = nc.dram_tensor(
        "wt", [M_PER_CORE, DUOS, PART, CH * PART], f16, kind="ExternalInput"
    )
    shf = nc.dram_tensor(
        "shf", [M_PER_CORE, DUOS, PART, SPF], f16, kind="ExternalInput"
    )
    out = nc.dram_tensor(
        "out", [M_PER_CORE, DUOS, PART, SPF], f16, kind="ExternalOutput"
    )

    with tile.TileContext(nc) as tc:
        with (
            tc.tile_pool(name="persist", bufs=1) as persist,
            tc.tile_pool(name="io", bufs=2) as io,
            tc.tile_pool(name="ps", bufs=1, space="PSUM") as pspool,
        ):
            k1t = [
                persist.tile([PART, NB, KS, WPAD], f16, tag=f"k1t{s}",
                             name=f"k1t{s}")
                for s in range(2)
            ]
            # weight tile free layout (chunk, col112): lhsT per chunk is
            # the 2D slice [:, c, :]
            wtl = [
                persist.tile([PART, CH, PART], f16, tag=f"wt{s}",
                             name=f"wt{s}")
                for s in range(2)
            ]
            psum = [
                pspool.tile([128, 512], f32, tag=f"pp{i}", name=f"pp{i}")
                for i in range(4)
            ]

            idx = 0
            for m in range(M_PER_CORE):
                for d in range(DUOS):
                    s = idx % 2
                    nc.sync.dma_start(
                        out=k1t[s].rearrange("c p h w -> c (p h w)"),
                        in_=k1r[m, d, :, :],
                    )
                    nc.gpsimd.dma_start(
                        out=wtl[s].rearrange("c a b -> c (a b)"),
                        in_=wt[m, d, :, :],
                    )
                    sh = io.tile([PART, SPF], f16, tag="shf", name="shf")
                    nc.scalar.dma_start(out=sh[:, :], in_=shf[m, d, :, :])

                    P = psum[idx % 4]
                    for ci, (kh, kw) in enumerate(CHUNKS):
                        oh0, oh1 = max(0, 3 - kh), min(KS, 10 - kh)
                        dst = P[0:PART, oh0 * 56:oh1 * 56]
                        lhsT = wtl[s][:, kh * KS + kw, :]
                        rhs = k1t[s][
                            :, :, oh0 + kh - 3:oh1 + kh - 3, kw:kw + KS
                        ].transpose([0, 2, 1, 3])
                        nc.tensor.matmul(
                            dst, lhsT, rhs,
                            start=(ci == 0), stop=(ci == CH - 1),
                        )

                    ost = io.tile([PART, SPF], f16, tag="ost", name="ost")
                    nc.vector.tensor_mul(
                        ost[:, :], P[0:PART, 0:SPF], sh[:, :]
                    )
                    nc.sync.dma_start(out=out[m, d, :, :], in_=ost[:, :])
                    idx += 1
    nc.compile()
    return nc


def _get_nc(mode=None):
    if mode is None:
        mode = MODE
    if mode not in _CACHE:
        _CACHE[mode] = _build_nc(mode)
    return _CACHE[mode]


def _prep(k1, k2, shell, factor):
    """Host-side input packing (per-core slices are views of these)."""
    k1 = np.asarray(k1, np.float32).reshape(16, NB, 16, NB, KS, KS, KS)
    k2 = np.asarray(k2, np.float32).reshape(16, NB, 16, NB, KS, KS, KS)
    shell = np.asarray(shell, np.float32).reshape(16, NB, 16, NB, KS, KS, KS)
    f = np.float32(np.asarray(factor).reshape(-1)[0])

    k1h = k1.astype(np.float16)   # [m, p, n, j, d, h, w]
    k2h = k2.astype(np.float16)   # [m, q, n, j, kd, kh, kw]

    # k1r: [m, n, j, id, p, ih, iw(13 padded)] -> (16, 8, 112, 728)
    k1r = np.zeros((16, 16, NB, KS, NB, KS, WPAD), np.float16)
    k1r[..., 3:10] = k1h.transpose(0, 2, 3, 4, 1, 5, 6)
    k1r = np.ascontiguousarray(k1r).reshape(16, DUOS, 2 * NB * KS, NB * KS * WPAD)

    # wt (block-diag Toeplitz): [m, duo, (nb,j,id)=112, (kh,kw)=49,
    # (nb',q,od)=112] with the nb==nb' diagonal blocks holding
    # k2[q, j, id-od+3, kh, kw] and zeros elsewhere
    wt = np.zeros((16, DUOS, 2, NB, KS, KS, KS, 2, NB, KS), np.float16)
    # k2p: [m, duo, nb, j, kd, kh, kw, q]
    k2p = k2h.transpose(0, 2, 3, 4, 5, 6, 1).reshape(
        16, DUOS, 2, NB, KS, KS, KS, NB
    )
    for nb in range(2):
        for kd in range(KS):
            for od in range(max(0, 3 - kd), min(KS, 10 - kd)):
                wt[:, :, nb, :, od + kd - 3, :, :, nb, :, od] = \
                    k2p[:, :, nb, :, kd]
    wt = np.ascontiguousarray(wt).reshape(16, DUOS, PART, CH * PART)

    # shf: shell*factor as [m, n, q, od, oh, p, ow] -> (16, 8, 112, 392)
    sh = (shell * f).astype(np.float16).transpose(0, 2, 3, 4, 5, 1, 6)
    sh = np.ascontiguousarray(sh).reshape(16, DUOS, 2 * NB * KS, SPF)

    return k1r, wt, sh


def _make_in_maps(k1, k2, shell, factor):
    k1r, wt, sh = _prep(k1, k2, shell, factor)
    maps = []
    for c in range(N_CORES):
        mlo = c * M_PER_CORE
        maps.append({
            "k1r": np.ascontiguousarray(k1r[mlo:mlo + M_PER_CORE]),
            "wt": np.ascontiguousarray(wt[mlo:mlo + M_PER_CORE]),
            "shf": np.ascontiguousarray(sh[mlo:mlo + M_PER_CORE]),
        })
    return maps


def _gather(results):
    outs = [np.asarray(r["out"]) for r in results]
    full = np.concatenate(outs, axis=0)  # (16, 8, 112, 392) fp16
    full = full.reshape(16, DUOS, 2, NB, KS, KS, NB, KS)
    # [m, duo, nb, q, od, oh, p, ow] -> [m, p, duo, nb, q, od, oh, ow]
    full = full.transpose(0, 6, 1, 2, 3, 4, 5, 7)
    return np.ascontiguousarray(full).reshape(128, 128, KS, KS, KS).astype(
        np.float32
    )


def kernel(k1, k2, shell, factor, _trace=False):
    from concourse.bass_utils import run_bass_kernel_spmd

    nc = _get_nc(MODE)
    in_maps = _make_in_maps(k1, k2, shell, factor)
    try:
        res = run_bass_kernel_spmd(
            nc, in_maps, core_ids=list(range(N_CORES)), trace=_trace
        )
    except ModuleNotFoundError:
        res = run_bass_kernel_spmd(
            nc, in_maps, core_ids=list(range(N_CORES)), trace=False
        )
    out = _gather(res.results)
    if _trace:
        return out, res
    return out


# revision 13
# speedup vs baseline: 54.4264x; 1.1435x over previous
"""Trainium2 Bass kernel for nn_ComposedCliffordSteerableKernel.

Computation (see reference): for each of 16x16 (m, n) block pairs, a tiny
3D conv (8,8,7^3) x (8,8,7^3) -> (8,8,7^3) with SAME padding, then
elementwise * shell * factor:

  out[m8+p, n8+q, od,oh,ow] =
      sum_{j,kd,kh,kw} k2[m8+q, n8+j, kd,kh,kw]
                     * k1[m8+p, n8+j, od+kd-3, oh+kh-3, ow+kw-3]

The cost model charges a matmul `output_free_size * cycles_per_row`
regardless of how many PE rows/columns are used, so the winning layout
maximizes contraction+output partitions per instruction and minimizes
streamed rows.  This kernel uses a *Toeplitz-in-depth* packing:

- PSUM partitions   = (nb, q, od)  : pair-in-duo, out blade, out depth = 112
- contraction rows  = (nb, j, id)  : pair-in-duo, in blade, abs. in depth = 112
- chunk loop        = (kh, kw)     : 49 accumulating matmuls per (m, duo)
- streamed free dim = (oh, p, ow)  : <= 392, oh restricted to the valid
                      window per kh (sum_kh win(kh) = 37 instead of 49)

The kd contraction is absorbed into a host-precomputed block-diagonal
Toeplitz weight tile w[(nb,j,id),(nb,q,od)] = k2[q,j,id-od+3,kh,kw]
(zero off the n-diagonal and off the |id-od|<=3 band).  rhs is plain k1
with (nb,j,id) on partitions and (p,ih,iw) in-partition (w zero-padded to
13 so iw=ow+kw-3 is always in range; ih stays interior thanks to the oh
window).  Chunk (kh=3,kw=3) runs first: its oh window is full, so the
accumulation group's start=True matmul covers the whole PSUM tile.

fp16 operands (measured ~3e-4 rel err vs the 2e-2 gate; PSUM accumulates
fp32).  shell*factor is folded host-side and applied during the PSUM
evacuation multiply; outputs return as fp16 and are unpacked on host.

Charged PE rows: 2m * 8duo * sum_{kh,kw} 8p*win(kh)*7ow = 232,064
(~97us at 2.4GHz) vs the previous 16-tile kernel's 12.9M (~5.4ms).

Sharding: core c takes output row-blocks 2c and 2c+1 (16 of 128 rows);
no inter-core communication.
"""

import sys

for _p in ("/opt/trn_rl_repo",):
    if _p not in sys.path:
        sys.path.insert(0, _p)

import numpy as np

NB = 8
KS = 7
N_CORES = 8
M_PER_CORE = 2
DUOS = 8                   # n-pair duos per m-block
PART = 112                 # (nb2, j8, id7) = (nb2, q8, od7)
SPF = KS * NB * KS         # 392 free: (oh, p, ow)
CH = KS * KS               # 49 (kh, kw) chunks

# chunk order: (3,3) first (full oh window -> start=True covers the
# whole psum tile), rest lexicographic; last chunk carries stop=True
CHUNKS = [(3, 3)] + [
    (kh, kw) for kh in range(KS) for kw in range(KS) if (kh, kw) != (3, 3)
]

MODE = "toep16"

_CACHE = {}


def _build_nc(mode):
    import concourse.tile as tile
    from concourse import bacc, mybir

    f16 = mybir.dt.float16
    f32 = mybir.dt.float32

    nc = bacc.Bacc("TRN2", target_bir_lowering=False, debug=False)

    k1r = nc.dram_tensor(
        "k1r", [M_PER_CORE, DUOS, PART, NB * KS * KS], f16,
        kind="ExternalInput"
    )
    # full block-diagonal Toeplitz incl. zeros: walrus requires a 2D
    # weights AP, so the op columns must be contiguous per chunk
    wt = nc.dram_tensor(
        "wt", [M_PER_CORE, DUOS, PART, CH * PART], f16, kind="ExternalInput"
    )
    shf = nc.dram_tensor(
        "shf", [M_PER_CORE, DUOS, PART, SPF], f16, kind="ExternalInput"
    )
    out = nc.dram_tensor(
        "out", [M_PER_CORE, DUOS, PART, SPF], f16, kind="ExternalOutput"
    )

    with tile.TileContext(nc) as tc:
        with (
            tc.tile_pool(name="persist", bufs=1) as persist,
            tc.tile_pool(name="io", bufs=2) as io,
            tc.tile_pool(name="ps", bufs=1, space="PSUM") as pspool,
        ):
            k1t = [
                persist.tile([PART, NB, KS, KS], f16, tag=f"k1t{s}",
                             name=f"k1t{s}")
                for s in range(2)
            ]
            # weight tile free layout (chunk, col112): lhsT per chunk is
            # the 2D slice [:, c, :]
            wtl = [
                persist.tile([PART, CH, PART], f16, tag=f"wt{s}",
                             name=f"wt{s}")
                for s in range(2)
            ]
            psum = [
                pspool.tile([128, 512], f32, tag=f"pp{i}", name=f"pp{i}")
                for i in range(4)
            ]

            idx = 0
            for m in range(M_PER_CORE):
                for d in range(DUOS):
                    s = idx % 2
                    nc.sync.dma_start(
                        out=k1t[s].rearrange("c p h w -> c (p h w)"),
                        in_=k1r[m, d, :, :],
                    )
                    nc.gpsimd.dma_start(
                        out=wtl[s].rearrange("c a b -> c (a b)"),
                        in_=wt[m, d, :, :],
                    )
                    sh = io.tile([PART, SPF], f16, tag="shf", name="shf")
                    nc.scalar.dma_start(out=sh[:, :], in_=shf[m, d, :, :])

                    P = psum[idx % 4]
                    Pv = P[0:PART, 0:SPF].rearrange(
                        "c (oh p ow) -> c oh p ow", oh=KS, p=NB
                    )
                    for ci, (kh, kw) in enumerate(CHUNKS):
                        oh0, oh1 = max(0, 3 - kh), min(KS, 10 - kh)
                        ow0, ow1 = max(0, 3 - kw), min(KS, 10 - kw)
                        dst = Pv[:, oh0:oh1, :, ow0:ow1]
                        lhsT = wtl[s][:, kh * KS + kw, :]
                        rhs = k1t[s][
                            :, :,
                            oh0 + kh - 3:oh1 + kh - 3,
                            ow0 + kw - 3:ow1 + kw - 3,
                        ].transpose([0, 2, 1, 3])
                        nc.tensor.matmul(
                            dst, lhsT, rhs,
                            start=(ci == 0), stop=(ci == CH - 1),
                        )

                    ost = io.tile([PART, SPF], f16, tag="ost", name="ost")
                    nc.vector.tensor_mul(
                        ost[:, :], P[0:PART, 0:SPF], sh[:, :]
                    )
                    nc.sync.dma_start(out=out[m, d, :, :], in_=ost[:, :])
                    idx += 1
    nc.compile()
    return nc


def _get_nc(mode=None):
    if mode is None:
        mode = MODE
    if mode not in _CACHE:
        _CACHE[mode] = _build_nc(mode)
    return _CACHE[mode]


def _prep(k1, k2, shell, factor):
    """Host-side input packing (per-core slices are views of these)."""
    k1 = np.asarray(k1, np.float32).reshape(16, NB, 16, NB, KS, KS, KS)
    k2 = np.asarray(k2, np.float32).reshape(16, NB, 16, NB, KS, KS, KS)
    shell = np.asarray(shell, np.float32).reshape(16, NB, 16, NB, KS, KS, KS)
    f = np.float32(np.asarray(factor).reshape(-1)[0])

    k1h = k1.astype(np.float16)   # [m, p, n, j, d, h, w]
    k2h = k2.astype(np.float16)   # [m, q, n, j, kd, kh, kw]

    # k1r: [m, n, j, id, p, ih, iw] -> (16, 8, 112, 392); no halo padding:
    # the oh/ow windows keep ih/iw interior
    k1r = np.ascontiguousarray(
        k1h.transpose(0, 2, 3, 4, 1, 5, 6)
    ).reshape(16, DUOS, 2 * NB * KS, NB * KS * KS)

    # wt (block-diag Toeplitz): [m, duo, (nb,j,id)=112, (kh,kw)=49,
    # (nb',q,od)=112] with the nb==nb' diagonal blocks holding
    # k2[q, j, id-od+3, kh, kw] and zeros elsewhere
    wt = np.zeros((16, DUOS, 2, NB, KS, KS, KS, 2, NB, KS), np.float16)
    # k2p: [m, duo, nb, j, kd, kh, kw, q]
    k2p = k2h.transpose(0, 2, 3, 4, 5, 6, 1).reshape(
        16, DUOS, 2, NB, KS, KS, KS, NB
    )
    for nb in range(2):
        for kd in range(KS):
            for od in range(max(0, 3 - kd), min(KS, 10 - kd)):
                wt[:, :, nb, :, od + kd - 3, :, :, nb, :, od] = \
                    k2p[:, :, nb, :, kd]
    wt = np.ascontiguousarray(wt).reshape(16, DUOS, PART, CH * PART)

    # shf: shell*factor as [m, n, q, od, oh, p, ow] -> (16, 8, 112, 392)
    sh = (shell * f).astype(np.float16).transpose(0, 2, 3, 4, 5, 1, 6)
    sh = np.ascontiguousarray(sh).reshape(16, DUOS, 2 * NB * KS, SPF)

    return k1r, wt, sh


def _make_in_maps(k1, k2, shell, factor):
    k1r, wt, sh = _prep(k1, k2, shell, factor)
    maps = []
    for c in range(N_CORES):
        mlo = c * M_PER_CORE
        maps.append({
            "k1r": np.ascontiguousarray(k1r[mlo:mlo + M_PER_CORE]),
            "wt": np.ascontiguousarray(wt[mlo:mlo + M_PER_CORE]),
            "shf": np.ascontiguousarray(sh[mlo:mlo + M_PER_CORE]),
        })
    return maps


def _gather(results):
    outs = [np.asarray(r["out"]) for r in results]
    full = np.concatenate(outs, axis=0)  # (16, 8, 112, 392) fp16
    full = full.reshape(16, DUOS, 2, NB, KS, KS, NB, KS)
    # [m, duo, nb, q, od, oh, p, ow] -> [m, p, duo, nb, q, od, oh, ow]
    full = full.transpose(0, 6, 1, 2, 3, 4, 5, 7)
    return np.ascontiguousarray(full).reshape(128, 128, KS, KS, KS).astype(
        np.float32
    )


def kernel(k1, k2, shell, factor, _trace=False):
    from concourse.bass_utils import run_bass_kernel_spmd

    nc = _get_nc(MODE)
    in_maps = _make_in_maps(k1, k2, shell, factor)
    try:
        res = run_bass_kernel_spmd(
            nc, in_maps, core_ids=list(range(N_CORES)), trace=_trace
        )
    except ModuleNotFoundError:
        res = run_bass_kernel_spmd(
            nc, in_maps, core_ids=list(range(N_CORES)), trace=False
        )
    out = _gather(res.results)
    if _trace:
        return out, res
    return out


# revision 21
# speedup vs baseline: 65.1019x; 1.1961x over previous
"""Trainium2 Bass kernel for nn_ComposedCliffordSteerableKernel.

Computation (see reference): for each of 16x16 (m, n) block pairs, a tiny
3D conv (8,8,7^3) x (8,8,7^3) -> (8,8,7^3) with SAME padding, then
elementwise * shell * factor:

  out[m8+p, n8+q, od,oh,ow] =
      sum_{j,kd,kh,kw} k2[m8+q, n8+j, kd,kh,kw]
                     * k1[m8+p, n8+j, od+kd-3, oh+kh-3, ow+kw-3]

The cost model charges a matmul `output_free_size * cycles_per_row`
regardless of how many PE rows/columns are used, so the winning layout
maximizes contraction+output partitions per instruction and minimizes
streamed rows.  This kernel uses a *Toeplitz-in-depth* packing:

- PSUM partitions   = (nb, q, od)  : pair-in-duo, out blade, out depth = 112
- contraction rows  = (nb, j, id)  : pair-in-duo, in blade, abs. in depth = 112
- chunk loop        = (kh, kw)     : 49 accumulating matmuls per (m, duo)
- streamed free dim = (oh, p, ow)  : <= 392, oh restricted to the valid
                      window per kh (sum_kh win(kh) = 37 instead of 49)

The kd contraction is absorbed into a host-precomputed block-diagonal
Toeplitz weight tile w[(nb,j,id),(nb,q,od)] = k2[q,j,id-od+3,kh,kw]
(zero off the n-diagonal and off the |id-od|<=3 band).  rhs is plain k1
with (nb,j,id) on partitions and (p,ih,iw) in-partition (w zero-padded to
13 so iw=ow+kw-3 is always in range; ih stays interior thanks to the oh
window).  Chunk (kh=3,kw=3) runs first: its oh window is full, so the
accumulation group's start=True matmul covers the whole PSUM tile.

fp16 operands (measured ~3e-4 rel err vs the 2e-2 gate; PSUM accumulates
fp32).  shell*factor is folded host-side and applied during the PSUM
evacuation multiply; outputs return as fp16 and are unpacked on host.

Charged PE rows: 2m * 8duo * sum_{kh,kw} 8p*win(kh)*7ow = 232,064
(~97us at 2.4GHz) vs the previous 16-tile kernel's 12.9M (~5.4ms).

Sharding: core c takes output row-blocks 2c and 2c+1 (16 of 128 rows);
no inter-core communication.
"""

import sys

for _p in ("/opt/trn_rl_repo",):
    if _p not in sys.path:
        sys.path.insert(0, _p)

import numpy as np

NB = 8
KS = 7
N_CORES = 8
M_PER_CORE = 2
DUOS = 8                   # n-pair duos per m-block
PART = 112                 # (nb2, j8, id7) = (nb2, q8, od7)
SPF = KS * NB * KS         # 392 free: (oh, p, ow)
CH = KS * KS               # 49 (kh, kw) chunks

# chunk order: (3,3) first (full oh window -> start=True covers the
# whole psum tile), rest lexicographic; last chunk carries stop=True.
# Host stores the weight chunks in THIS order so a small prefix DMA
# unblocks the first matmuls.
CHUNKS = [(3, 3)] + [
    (kh, kw) for kh in range(KS) for kw in range(KS) if (kh, kw) != (3, 3)
]
WPRE = 8                   # chunks in the prefix weight DMA
NSLOT = 4                  # k1/weight buffer slots (DMA prefetch depth)

MODE = "toep16"

_CACHE = {}


def _build_nc(mode):
    import concourse.tile as tile
    from concourse import bacc, mybir

    f16 = mybir.dt.float16
    f32 = mybir.dt.float32

    nc = bacc.Bacc("TRN2", target_bir_lowering=False, debug=False)

    k1r = nc.dram_tensor(
        "k1r", [M_PER_CORE, DUOS, PART, NB * KS * KS], f16,
        kind="ExternalInput"
    )
    # full block-diagonal Toeplitz incl. zeros: walrus requires a 2D
    # weights AP, so the op columns must be contiguous per chunk
    wt = nc.dram_tensor(
        "wt", [M_PER_CORE, DUOS, PART, CH * PART], f16, kind="ExternalInput"
    )
    shf = nc.dram_tensor(
        "shf", [M_PER_CORE, DUOS, PART, SPF], f16, kind="ExternalInput"
    )
    out = nc.dram_tensor(
        "out", [M_PER_CORE, DUOS, PART, SPF], f16, kind="ExternalOutput"
    )

    with tile.TileContext(nc) as tc:
        with (
            tc.tile_pool(name="persist", bufs=1) as persist,
            tc.tile_pool(name="io", bufs=2) as io,
            tc.tile_pool(name="ps", bufs=1, space="PSUM") as pspool,
        ):
            k1t = [
                persist.tile([PART, NB, KS, KS], f16, tag=f"k1t{s}",
                             name=f"k1t{s}")
                for s in range(NSLOT)
            ]
            # weight tile free layout (chunk, col112): lhsT per chunk is
            # the 2D slice [:, c, :]
            wtl = [
                persist.tile([PART, CH, PART], f16, tag=f"wt{s}",
                             name=f"wt{s}")
                for s in range(NSLOT)
            ]
            psum = [
                pspool.tile([128, 512], f32, tag=f"pp{i}", name=f"pp{i}")
                for i in range(4)
            ]

            idx = 0
            for m in range(M_PER_CORE):
                for d in range(DUOS):
                    s = idx % NSLOT
                    nc.sync.dma_start(
                        out=k1t[s].rearrange("c p h w -> c (p h w)"),
                        in_=k1r[m, d, :, :],
                    )
                    # prefix first so the duo's first matmuls unblock
                    # before the bulk of the weights lands
                    nc.gpsimd.dma_start(
                        out=wtl[s][:, 0:WPRE, :].rearrange("c a b -> c (a b)"),
                        in_=wt[m, d, :, 0:WPRE * PART],
                    )
                    nc.gpsimd.dma_start(
                        out=wtl[s][:, WPRE:CH, :].rearrange("c a b -> c (a b)"),
                        in_=wt[m, d, :, WPRE * PART:CH * PART],
                    )
                    sh = io.tile([PART, SPF], f16, tag="shf", name="shf")
                    nc.sync.dma_start(out=sh[:, :], in_=shf[m, d, :, :])

                    P = psum[idx % 4]
                    Pv = P[0:PART, 0:SPF].rearrange(
                        "c (oh p ow) -> c oh p ow", oh=KS, p=NB
                    )
                    for ci, (kh, kw) in enumerate(CHUNKS):
                        oh0, oh1 = max(0, 3 - kh), min(KS, 10 - kh)
                        ow0, ow1 = max(0, 3 - kw), min(KS, 10 - kw)
                        dst = Pv[:, oh0:oh1, :, ow0:ow1]
                        lhsT = wtl[s][:, ci, :]
                        rhs = k1t[s][
                            :, :,
                            oh0 + kh - 3:oh1 + kh - 3,
                            ow0 + kw - 3:ow1 + kw - 3,
                        ].transpose([0, 2, 1, 3])
                        nc.tensor.matmul(
                            dst, lhsT, rhs,
                            start=(ci == 0), stop=(ci == CH - 1),
                        )

                    ost = io.tile([PART, SPF], f16, tag="ost", name="ost")
                    nc.vector.tensor_mul(
                        ost[:, :], P[0:PART, 0:SPF], sh[:, :]
                    )
                    # out gets the Activation queue to itself: its SEQ-stage
                    # wait on the evacuation would block k1/shell prefetch
                    # if it shared SP, or weight prefetch if it shared Pool
                    nc.scalar.dma_start(out=out[m, d, :, :], in_=ost[:, :])
                    idx += 1
    nc.compile()
    return nc


def _get_nc(mode=None):
    if mode is None:
        mode = MODE
    if mode not in _CACHE:
        _CACHE[mode] = _build_nc(mode)
    return _CACHE[mode]


def _prep(k1, k2, shell, factor):
    """Host-side input packing (per-core slices are views of these)."""
    k1 = np.asarray(k1, np.float32).reshape(16, NB, 16, NB, KS, KS, KS)
    k2 = np.asarray(k2, np.float32).reshape(16, NB, 16, NB, KS, KS, KS)
    shell = np.asarray(shell, np.float32).reshape(16, NB, 16, NB, KS, KS, KS)
    f = np.float32(np.asarray(factor).reshape(-1)[0])

    k1h = k1.astype(np.float16)   # [m, p, n, j, d, h, w]
    k2h = k2.astype(np.float16)   # [m, q, n, j, kd, kh, kw]

    # k1r: [m, n, j, id, p, ih, iw] -> (16, 8, 112, 392); no halo padding:
    # the oh/ow windows keep ih/iw interior
    k1r = np.ascontiguousarray(
        k1h.transpose(0, 2, 3, 4, 1, 5, 6)
    ).reshape(16, DUOS, 2 * NB * KS, NB * KS * KS)

    # wt (block-diag Toeplitz): [m, duo, (nb,j,id)=112, chunk=49,
    # (nb',q,od)=112] with the nb==nb' diagonal blocks holding
    # k2[q, j, id-od+3, kh, kw] and zeros elsewhere; the chunk axis is
    # stored in CHUNKS (issue) order
    wt = np.zeros((16, DUOS, 2, NB, KS, CH, 2, NB, KS), np.float16)
    # k2p: [m, duo, nb, j, kd, kh, kw, q]
    k2p = k2h.transpose(0, 2, 3, 4, 5, 6, 1).reshape(
        16, DUOS, 2, NB, KS, KS, KS, NB
    )
    for ci, (kh, kw) in enumerate(CHUNKS):
        for nb in range(2):
            for kd in range(KS):
                for od in range(max(0, 3 - kd), min(KS, 10 - kd)):
                    wt[:, :, nb, :, od + kd - 3, ci, nb, :, od] = \
                        k2p[:, :, nb, :, kd, kh, kw]
    wt = np.ascontiguousarray(wt).reshape(16, DUOS, PART, CH * PART)

    # shf: shell*factor as [m, n, q, od, oh, p, ow] -> (16, 8, 112, 392)
    sh = (shell * f).astype(np.float16).transpose(0, 2, 3, 4, 5, 1, 6)
    sh = np.ascontiguousarray(sh).reshape(16, DUOS, 2 * NB * KS, SPF)

    return k1r, wt, sh


def _make_in_maps(k1, k2, shell, factor):
    k1r, wt, sh = _prep(k1, k2, shell, factor)
    maps = []
    for c in range(N_CORES):
        mlo = c * M_PER_CORE
        maps.append({
            "k1r": np.ascontiguousarray(k1r[mlo:mlo + M_PER_CORE]),
            "wt": np.ascontiguousarray(wt[mlo:mlo + M_PER_CORE]),
            "shf": np.ascontiguousarray(sh[mlo:mlo + M_PER_CORE]),
        })
    return maps


def _gather(results):
    outs = [np.asarray(r["out"]) for r in results]
    full = np.concatenate(outs, axis=0)  # (16, 8, 112, 392) fp16
    full = full.reshape(16, DUOS, 2, NB, KS, KS, NB, KS)
    # [m, duo, nb, q, od, oh, p, ow] -> [m, p, duo, nb, q, od, oh, ow]
    full = full.transpose(0, 6, 1, 2, 3, 4, 5, 7)
    return np.ascontiguousarray(full).reshape(128, 128, KS, KS, KS).astype(
        np.float32
    )


def kernel(k1, k2, shell, factor, _trace=False):
    from concourse.bass_utils import run_bass_kernel_spmd

    nc = _get_nc(MODE)
    in_maps = _make_in_maps(k1, k2, shell, factor)
    try:
        res = run_bass_kernel_spmd(
            nc, in_maps, core_ids=list(range(N_CORES)), trace=_trace
        )
    except ModuleNotFoundError:
        res = run_bass_kernel_spmd(
            nc, in_maps, core_ids=list(range(N_CORES)), trace=False
        )
    out = _gather(res.results)
    if _trace:
        return out, res
    return out


# revision 23
# speedup vs baseline: 65.6316x; 1.0081x over previous
"""Trainium2 Bass kernel for nn_ComposedCliffordSteerableKernel.

Computation (see reference): for each of 16x16 (m, n) block pairs, a tiny
3D conv (8,8,7^3) x (8,8,7^3) -> (8,8,7^3) with SAME padding, then
elementwise * shell * factor:

  out[m8+p, n8+q, od,oh,ow] =
      sum_{j,kd,kh,kw} k2[m8+q, n8+j, kd,kh,kw]
                     * k1[m8+p, n8+j, od+kd-3, oh+kh-3, ow+kw-3]

The cost model charges a matmul `output_free_size * cycles_per_row`
regardless of how many PE rows/columns are used, so the winning layout
maximizes contraction+output partitions per instruction and minimizes
streamed rows.  This kernel uses a *Toeplitz-in-depth* packing:

- PSUM partitions   = (nb, q, od)  : pair-in-duo, out blade, out depth = 112
- contraction rows  = (nb, j, id)  : pair-in-duo, in blade, abs. in depth = 112
- chunk loop        = (kh, kw)     : 49 accumulating matmuls per (m, duo)
- streamed free dim = (oh, p, ow)  : <= 392, oh restricted to the valid
                      window per kh (sum_kh win(kh) = 37 instead of 49)

The kd contraction is absorbed into a host-precomputed block-diagonal
Toeplitz weight tile w[(nb,j,id),(nb,q,od)] = k2[q,j,id-od+3,kh,kw]
(zero off the n-diagonal and off the |id-od|<=3 band).  rhs is plain k1
with (nb,j,id) on partitions and (p,ih,iw) in-partition (w zero-padded to
13 so iw=ow+kw-3 is always in range; ih stays interior thanks to the oh
window).  Chunk (kh=3,kw=3) runs first: its oh window is full, so the
accumulation group's start=True matmul covers the whole PSUM tile.

fp16 operands (measured ~3e-4 rel err vs the 2e-2 gate; PSUM accumulates
fp32).  shell*factor is folded host-side and applied during the PSUM
evacuation multiply; outputs return as fp16 and are unpacked on host.

Charged PE rows: 2m * 8duo * sum_{kh,kw} 8p*win(kh)*7ow = 232,064
(~97us at 2.4GHz) vs the previous 16-tile kernel's 12.9M (~5.4ms).

Sharding: core c takes output row-blocks 2c and 2c+1 (16 of 128 rows);
no inter-core communication.
"""

import sys

for _p in ("/opt/trn_rl_repo",):
    if _p not in sys.path:
        sys.path.insert(0, _p)

import numpy as np

NB = 8
KS = 7
N_CORES = 8
M_PER_CORE = 2
DUOS = 8                   # n-pair duos per m-block
PART = 112                 # (nb2, j8, id7) = (nb2, q8, od7)
SPF = KS * NB * KS         # 392 free: (oh, p, ow)
CH = KS * KS               # 49 (kh, kw) chunks

# chunk order: (3,3) first (full oh window -> start=True covers the
# whole psum tile), rest lexicographic; last chunk carries stop=True.
# Host stores the weight chunks in THIS order so a small prefix DMA
# unblocks the first matmuls.
CHUNKS = [(3, 3)] + [
    (kh, kw) for kh in range(KS) for kw in range(KS) if (kh, kw) != (3, 3)
]
WSPLIT = (6, 12, 31)       # weight DMA chunk split (startup pipelining)
NSLOT = 4                  # k1/weight buffer slots (DMA prefetch depth)

MODE = "toep16"

_CACHE = {}


def _build_nc(mode):
    import concourse.tile as tile
    from concourse import bacc, mybir

    f16 = mybir.dt.float16
    f32 = mybir.dt.float32

    nc = bacc.Bacc("TRN2", target_bir_lowering=False, debug=False)

    k1r = nc.dram_tensor(
        "k1r", [M_PER_CORE, DUOS, PART, NB * KS * KS], f16,
        kind="ExternalInput"
    )
    # full block-diagonal Toeplitz incl. zeros: walrus requires a 2D
    # weights AP, so the op columns must be contiguous per chunk
    wt = nc.dram_tensor(
        "wt", [M_PER_CORE, DUOS, PART, CH * PART], f16, kind="ExternalInput"
    )
    shf = nc.dram_tensor(
        "shf", [M_PER_CORE, DUOS, PART, SPF], f16, kind="ExternalInput"
    )
    out = nc.dram_tensor(
        "out", [M_PER_CORE, DUOS, PART, SPF], f16, kind="ExternalOutput"
    )

    with tile.TileContext(nc) as tc:
        with (
            tc.tile_pool(name="persist", bufs=1) as persist,
            tc.tile_pool(name="io", bufs=2) as io,
            tc.tile_pool(name="ps", bufs=1, space="PSUM") as pspool,
        ):
            k1t = [
                persist.tile([PART, NB, KS, KS], f16, tag=f"k1t{s}",
                             name=f"k1t{s}")
                for s in range(NSLOT)
            ]
            # weight tile free layout (chunk, col112): lhsT per chunk is
            # the 2D slice [:, c, :]
            wtl = [
                persist.tile([PART, CH, PART], f16, tag=f"wt{s}",
                             name=f"wt{s}")
                for s in range(NSLOT)
            ]
            psum = [
                pspool.tile([128, 512], f32, tag=f"pp{i}", name=f"pp{i}")
                for i in range(4)
            ]

            idx = 0
            for m in range(M_PER_CORE):
                for d in range(DUOS):
                    s = idx % NSLOT
                    nc.sync.dma_start(
                        out=k1t[s].rearrange("c p h w -> c (p h w)"),
                        in_=k1r[m, d, :, :],
                    )
                    # weights land in issue-order pieces so the duo's
                    # early matmuls unblock before the bulk arrives
                    c0 = 0
                    for npiece in WSPLIT:
                        c1 = c0 + npiece
                        nc.gpsimd.dma_start(
                            out=wtl[s][:, c0:c1, :].rearrange(
                                "c a b -> c (a b)"
                            ),
                            in_=wt[m, d, :, c0 * PART:c1 * PART],
                        )
                        c0 = c1
                    sh = io.tile([PART, SPF], f16, tag="shf", name="shf")
                    nc.sync.dma_start(out=sh[:, :], in_=shf[m, d, :, :])

                    P = psum[idx % 4]
                    Pv = P[0:PART, 0:SPF].rearrange(
                        "c (oh p ow) -> c oh p ow", oh=KS, p=NB
                    )
                    for ci, (kh, kw) in enumerate(CHUNKS):
                        oh0, oh1 = max(0, 3 - kh), min(KS, 10 - kh)
                        ow0, ow1 = max(0, 3 - kw), min(KS, 10 - kw)
                        dst = Pv[:, oh0:oh1, :, ow0:ow1]
                        lhsT = wtl[s][:, ci, :]
                        rhs = k1t[s][
                            :, :,
                            oh0 + kh - 3:oh1 + kh - 3,
                            ow0 + kw - 3:ow1 + kw - 3,
                        ].transpose([0, 2, 1, 3])
                        nc.tensor.matmul(
                            dst, lhsT, rhs,
                            start=(ci == 0), stop=(ci == CH - 1),
                        )

                    ost = io.tile([PART, SPF], f16, tag="ost", name="ost")
                    nc.vector.tensor_mul(
                        ost[:, :], P[0:PART, 0:SPF], sh[:, :]
                    )
                    # out gets the Activation queue to itself: its SEQ-stage
                    # wait on the evacuation would block k1/shell prefetch
                    # if it shared SP, or weight prefetch if it shared Pool
                    nc.scalar.dma_start(out=out[m, d, :, :], in_=ost[:, :])
                    idx += 1
    nc.compile()
    return nc


def _get_nc(mode=None):
    if mode is None:
        mode = MODE
    if mode not in _CACHE:
        _CACHE[mode] = _build_nc(mode)
    return _CACHE[mode]


def _prep(k1, k2, shell, factor):
    """Host-side input packing (per-core slices are views of these)."""
    k1 = np.asarray(k1, np.float32).reshape(16, NB, 16, NB, KS, KS, KS)
    k2 = np.asarray(k2, np.float32).reshape(16, NB, 16, NB, KS, KS, KS)
    shell = np.asarray(shell, np.float32).reshape(16, NB, 16, NB, KS, KS, KS)
    f = np.float32(np.asarray(factor).reshape(-1)[0])

    k1h = k1.astype(np.float16)   # [m, p, n, j, d, h, w]
    k2h = k2.astype(np.float16)   # [m, q, n, j, kd, kh, kw]

    # k1r: [m, n, j, id, p, ih, iw] -> (16, 8, 112, 392); no halo padding:
    # the oh/ow windows keep ih/iw interior
    k1r = np.ascontiguousarray(
        k1h.transpose(0, 2, 3, 4, 1, 5, 6)
    ).reshape(16, DUOS, 2 * NB * KS, NB * KS * KS)

    # wt (block-diag Toeplitz): [m, duo, (nb,j,id)=112, chunk=49,
    # (nb',q,od)=112] with the nb==nb' diagonal blocks holding
    # k2[q, j, id-od+3, kh, kw] and zeros elsewhere; the chunk axis is
    # stored in CHUNKS (issue) order
    wt = np.zeros((16, DUOS, 2, NB, KS, CH, 2, NB, KS), np.float16)
    # k2p: [m, duo, nb, j, kd, kh, kw, q]
    k2p = k2h.transpose(0, 2, 3, 4, 5, 6, 1).reshape(
        16, DUOS, 2, NB, KS, KS, KS, NB
    )
    for ci, (kh, kw) in enumerate(CHUNKS):
        for nb in range(2):
            for kd in range(KS):
                for od in range(max(0, 3 - kd), min(KS, 10 - kd)):
                    wt[:, :, nb, :, od + kd - 3, ci, nb, :, od] = \
                        k2p[:, :, nb, :, kd, kh, kw]
    wt = np.ascontiguousarray(wt).reshape(16, DUOS, PART, CH * PART)

    # shf: shell*factor as [m, n, q, od, oh, p, ow] -> (16, 8, 112, 392)
    sh = (shell * f).astype(np.float16).transpose(0, 2, 3, 4, 5, 1, 6)
    sh = np.ascontiguousarray(sh).reshape(16, DUOS, 2 * NB * KS, SPF)

    return k1r, wt, sh


def _make_in_maps(k1, k2, shell, factor):
    k1r, wt, sh = _prep(k1, k2, shell, factor)
    maps = []
    for c in range(N_CORES):
        mlo = c * M_PER_CORE
        maps.append({
            "k1r": np.ascontiguousarray(k1r[mlo:mlo + M_PER_CORE]),
            "wt": np.ascontiguousarray(wt[mlo:mlo + M_PER_CORE]),
            "shf": np.ascontiguousarray(sh[mlo:mlo + M_PER_CORE]),
        })
    return maps


def _gather(results):
    outs = [np.asarray(r["out"]) for r in results]
    full = np.concatenate(outs, axis=0)  # (16, 8, 112, 392) fp16
    full = full.reshape(16, DUOS, 2, NB, KS, KS, NB, KS)
    # [m, duo, nb, q, od, oh, p, ow] -> [m, p, duo, nb, q, od, oh, ow]
    full = full.transpose(0, 6, 1, 2, 3, 4, 5, 7)
    return np.ascontiguousarray(full).reshape(128, 128, KS, KS, KS).astype(
        np.float32
    )


def kernel(k1, k2, shell, factor, _trace=False):
    from concourse.bass_utils import run_bass_kernel_spmd

    nc = _get_nc(MODE)
    in_maps = _make_in_maps(k1, k2, shell, factor)
    try:
        res = run_bass_kernel_spmd(
            nc, in_maps, core_ids=list(range(N_CORES)), trace=_trace
        )
    except ModuleNotFoundError:
        res = run_bass_kernel_spmd(
            nc, in_maps, core_ids=list(range(N_CORES)), trace=False
        )
    out = _gather(res.results)
    if _trace:
        return out, res
    return out


# revision 28
# speedup vs baseline: 65.7002x; 1.0010x over previous
"""Trainium2 Bass kernel for nn_ComposedCliffordSteerableKernel.

Computation (see reference): for each of 16x16 (m, n) block pairs, a tiny
3D conv (8,8,7^3) x (8,8,7^3) -> (8,8,7^3) with SAME padding, then
elementwise * shell * factor:

  out[m8+p, n8+q, od,oh,ow] =
      sum_{j,kd,kh,kw} k2[m8+q, n8+j, kd,kh,kw]
                     * k1[m8+p, n8+j, od+kd-3, oh+kh-3, ow+kw-3]

The cost model charges a matmul `output_free_size * cycles_per_row`
regardless of how many PE rows/columns are used, so the winning layout
maximizes contraction+output partitions per instruction and minimizes
streamed rows.  This kernel uses a *Toeplitz-in-depth* packing:

- PSUM partitions   = (nb, q, od)  : pair-in-duo, out blade, out depth = 112
- contraction rows  = (nb, j, id)  : pair-in-duo, in blade, abs. in depth = 112
- chunk loop        = (kh, kw)     : 49 accumulating matmuls per (m, duo)
- streamed free dim = (oh, p, ow)  : <= 392, oh restricted to the valid
                      window per kh (sum_kh win(kh) = 37 instead of 49)

The kd contraction is absorbed into a host-precomputed block-diagonal
Toeplitz weight tile w[(nb,j,id),(nb,q,od)] = k2[q,j,id-od+3,kh,kw]
(zero off the n-diagonal and off the |id-od|<=3 band).  rhs is plain k1
with (nb,j,id) on partitions and (p,ih,iw) in-partition (w zero-padded to
13 so iw=ow+kw-3 is always in range; ih stays interior thanks to the oh
window).  Chunk (kh=3,kw=3) runs first: its oh window is full, so the
accumulation group's start=True matmul covers the whole PSUM tile.

fp16 operands (measured ~3e-4 rel err vs the 2e-2 gate; PSUM accumulates
fp32).  shell*factor is folded host-side and applied during the PSUM
evacuation multiply; outputs return as fp16 and are unpacked on host.

Charged PE rows: 2m * 8duo * sum_{kh,kw} 8p*win(kh)*7ow = 232,064
(~97us at 2.4GHz) vs the previous 16-tile kernel's 12.9M (~5.4ms).

Sharding: core c takes output row-blocks 2c and 2c+1 (16 of 128 rows);
no inter-core communication.
"""

import sys

for _p in ("/opt/trn_rl_repo",):
    if _p not in sys.path:
        sys.path.insert(0, _p)

import numpy as np

NB = 8
KS = 7
N_CORES = 8
M_PER_CORE = 2
DUOS = 8                   # n-pair duos per m-block
PART = 112                 # (nb2, j8, id7) = (nb2, q8, od7)
SPF = KS * NB * KS         # 392 free: (p, oh, ow)
NWARM = 7                  # PE warm-up matmuls (bridge the p-state ramp)
CH = KS * KS               # 49 (kh, kw) chunks

# chunk order: (3,3) first (full oh window -> start=True covers the
# whole psum tile), rest lexicographic; last chunk carries stop=True.
# Host stores the weight chunks in THIS order so a small prefix DMA
# unblocks the first matmuls.
CHUNKS = [(3, 3)] + [
    (kh, kw) for kh in range(KS) for kw in range(KS) if (kh, kw) != (3, 3)
]
WSPLIT = (6, 12, 31)       # weight DMA chunk split (startup pipelining)
NSLOT = 4                  # k1/weight buffer slots (DMA prefetch depth)

MODE = "toep16"

_CACHE = {}


def _build_nc(mode):
    import concourse.tile as tile
    from concourse import bacc, mybir

    f16 = mybir.dt.float16
    f32 = mybir.dt.float32

    nc = bacc.Bacc("TRN2", target_bir_lowering=False, debug=False)

    k1r = nc.dram_tensor(
        "k1r", [M_PER_CORE, DUOS, PART, NB * KS * KS], f16,
        kind="ExternalInput"
    )
    # full block-diagonal Toeplitz incl. zeros: walrus requires a 2D
    # weights AP, so the op columns must be contiguous per chunk
    wt = nc.dram_tensor(
        "wt", [M_PER_CORE, DUOS, PART, CH * PART], f16, kind="ExternalInput"
    )
    shf = nc.dram_tensor(
        "shf", [M_PER_CORE, DUOS, PART, SPF], f16, kind="ExternalInput"
    )
    out = nc.dram_tensor(
        "out", [M_PER_CORE, DUOS, PART, SPF], f16, kind="ExternalOutput"
    )

    with tile.TileContext(nc) as tc:
        with (
            tc.tile_pool(name="persist", bufs=1) as persist,
            tc.tile_pool(name="io", bufs=2) as io,
            tc.tile_pool(name="ps", bufs=1, space="PSUM") as pspool,
        ):
            k1t = [
                persist.tile([PART, NB, KS, KS], f16, tag=f"k1t{s}",
                             name=f"k1t{s}")
                for s in range(NSLOT)
            ]
            # weight tile free layout (chunk, col112): lhsT per chunk is
            # the 2D slice [:, c, :]
            wtl = [
                persist.tile([PART, CH, PART], f16, tag=f"wt{s}",
                             name=f"wt{s}")
                for s in range(NSLOT)
            ]
            psum = [
                pspool.tile([128, 512], f32, tag=f"pp{i}", name=f"pp{i}")
                for i in range(4)
            ]
            # extra half-banks for the final duo's split evacuation
            phalf = [
                pspool.tile([128, 256], f32, tag=f"ph{i}", name=f"ph{i}")
                for i in range(2)
            ]

            # PE p-state warm-up: the PE runs at ~1/3..1/2 clock for the
            # first 3us of continuous busy.  Dummy matmuls on a scratch
            # bank during the initial DMA window put the ramp where the
            # PE would idle anyway, so real work starts at full clock.
            warm = persist.tile([PART, 512], f16, tag="warm", name="warm")
            nc.vector.memset(warm[:, :], 0.0)
            psw = pspool.tile([128, 512], f32, tag="psw", name="psw")
            for _ in range(NWARM):
                nc.tensor.matmul(
                    psw[0:PART, 0:512], warm[:, 0:PART], warm[:, 0:512],
                    start=True, stop=True,
                )

            idx = 0
            for m in range(M_PER_CORE):
                for d in range(DUOS):
                    s = idx % NSLOT
                    nc.sync.dma_start(
                        out=k1t[s].rearrange("c p h w -> c (p h w)"),
                        in_=k1r[m, d, :, :],
                    )
                    # weights land in issue-order pieces so the duo's
                    # early matmuls unblock before the bulk arrives
                    c0 = 0
                    for npiece in WSPLIT:
                        c1 = c0 + npiece
                        nc.gpsimd.dma_start(
                            out=wtl[s][:, c0:c1, :].rearrange(
                                "c a b -> c (a b)"
                            ),
                            in_=wt[m, d, :, c0 * PART:c1 * PART],
                        )
                        c0 = c1
                    sh = io.tile([PART, SPF], f16, tag="shf", name="shf")
                    nc.sync.dma_start(out=sh[:, :], in_=shf[m, d, :, :])

                    last = idx == M_PER_CORE * DUOS - 1
                    ost = io.tile([PART, SPF], f16, tag="ost", name="ost")
                    if not last:
                        halves = [(psum[idx % 4], 0, NB)]
                    else:
                        # final duo: accumulate the p-halves in separate
                        # banks, all of A's chunks before B's, so A's
                        # evacuation+store overlaps B's matmuls and only
                        # half a store remains after the last matmul
                        halves = [(phalf[0], 0, 4), (phalf[1], 4, NB)]
                    for P, p0, p1 in halves:
                        np_ = p1 - p0
                        fsz = np_ * KS * KS
                        Pv = P[0:PART, 0:fsz].rearrange(
                            "c (p oh ow) -> c p oh ow", p=np_, oh=KS
                        )
                        for ci, (kh, kw) in enumerate(CHUNKS):
                            oh0, oh1 = max(0, 3 - kh), min(KS, 10 - kh)
                            ow0, ow1 = max(0, 3 - kw), min(KS, 10 - kw)
                            dst = Pv[:, :, oh0:oh1, ow0:ow1]
                            lhsT = wtl[s][:, ci, :]
                            rhs = k1t[s][
                                :, p0:p1,
                                oh0 + kh - 3:oh1 + kh - 3,
                                ow0 + kw - 3:ow1 + kw - 3,
                            ]
                            nc.tensor.matmul(
                                dst, lhsT, rhs,
                                start=(ci == 0), stop=(ci == CH - 1),
                            )
                        f0, f1 = p0 * KS * KS, p1 * KS * KS
                        nc.vector.tensor_mul(
                            ost[:, f0:f1], P[0:PART, 0:fsz], sh[:, f0:f1]
                        )
                        # out gets the Activation queue to itself: its
                        # SEQ-stage wait on the evacuation would block
                        # k1/shell prefetch if it shared SP, or weight
                        # prefetch if it shared Pool
                        nc.scalar.dma_start(
                            out=out[m, d, :, f0:f1], in_=ost[:, f0:f1]
                        )
                    idx += 1
    nc.compile()
    return nc


def _get_nc(mode=None):
    if mode is None:
        mode = MODE
    if mode not in _CACHE:
        _CACHE[mode] = _build_nc(mode)
    return _CACHE[mode]


def _prep(k1, k2, shell, factor):
    """Host-side input packing (per-core slices are views of these)."""
    k1 = np.asarray(k1, np.float32).reshape(16, NB, 16, NB, KS, KS, KS)
    k2 = np.asarray(k2, np.float32).reshape(16, NB, 16, NB, KS, KS, KS)
    shell = np.asarray(shell, np.float32).reshape(16, NB, 16, NB, KS, KS, KS)
    f = np.float32(np.asarray(factor).reshape(-1)[0])

    k1h = k1.astype(np.float16)   # [m, p, n, j, d, h, w]
    k2h = k2.astype(np.float16)   # [m, q, n, j, kd, kh, kw]

    # k1r: [m, n, j, id, p, ih, iw] -> (16, 8, 112, 392); no halo padding:
    # the oh/ow windows keep ih/iw interior
    k1r = np.ascontiguousarray(
        k1h.transpose(0, 2, 3, 4, 1, 5, 6)
    ).reshape(16, DUOS, 2 * NB * KS, NB * KS * KS)

    # wt (block-diag Toeplitz): [m, duo, (nb,j,id)=112, chunk=49,
    # (nb',q,od)=112] with the nb==nb' diagonal blocks holding
    # k2[q, j, id-od+3, kh, kw] and zeros elsewhere; the chunk axis is
    # stored in CHUNKS (issue) order
    wt = np.zeros((16, DUOS, 2, NB, KS, CH, 2, NB, KS), np.float16)
    # k2p: [m, duo, nb, j, kd, kh, kw, q]
    k2p = k2h.transpose(0, 2, 3, 4, 5, 6, 1).reshape(
        16, DUOS, 2, NB, KS, KS, KS, NB
    )
    for ci, (kh, kw) in enumerate(CHUNKS):
        for nb in range(2):
            for kd in range(KS):
                for od in range(max(0, 3 - kd), min(KS, 10 - kd)):
                    wt[:, :, nb, :, od + kd - 3, ci, nb, :, od] = \
                        k2p[:, :, nb, :, kd, kh, kw]
    wt = np.ascontiguousarray(wt).reshape(16, DUOS, PART, CH * PART)

    # shf: shell*factor as [m, n, q, od, p, oh, ow] -> (16, 8, 112, 392)
    sh = (shell * f).astype(np.float16).transpose(0, 2, 3, 4, 1, 5, 6)
    sh = np.ascontiguousarray(sh).reshape(16, DUOS, 2 * NB * KS, SPF)

    return k1r, wt, sh


def _make_in_maps(k1, k2, shell, factor):
    k1r, wt, sh = _prep(k1, k2, shell, factor)
    maps = []
    for c in range(N_CORES):
        mlo = c * M_PER_CORE
        maps.append({
            "k1r": np.ascontiguousarray(k1r[mlo:mlo + M_PER_CORE]),
            "wt": np.ascontiguousarray(wt[mlo:mlo + M_PER_CORE]),
            "shf": np.ascontiguousarray(sh[mlo:mlo + M_PER_CORE]),
        })
    return maps


def _gather(results):
    outs = [np.asarray(r["out"]) for r in results]
    full = np.concatenate(outs, axis=0)  # (16, 8, 112, 392) fp16
    full = full.reshape(16, DUOS, 2, NB, KS, NB, KS, KS)
    # [m, duo, nb, q, od, p, oh, ow] -> [m, p, duo, nb, q, od, oh, ow]
    full = full.transpose(0, 5, 1, 2, 3, 4, 6, 7)
    return np.ascontiguousarray(full).reshape(128, 128, KS, KS, KS).astype(
        np.float32
    )


def kernel(k1, k2, shell, factor, _trace=False):
    from concourse.bass_utils import run_bass_kernel_spmd

    nc = _get_nc(MODE)
    in_maps = _make_in_maps(k1, k2, shell, factor)
    try:
        res = run_bass_kernel_spmd(
            nc, in_maps, core_ids=list(range(N_CORES)), trace=_trace
        )
    except ModuleNotFoundError:
        res = run_bass_kernel_spmd(
            nc, in_maps, core_ids=list(range(N_CORES)), trace=False
        )
    out = _gather(res.results)
    if _trace:
        return out, res
    return out
